# revision 5
# baseline (speedup 1.0000x reference)
"""LinearOffsetLayer Trainium2 kernel (8 NeuronCores, tensor-parallel on out_features).

Math:  A[o,i] = sum_d theta_d[d] * P_A[o,d,i] + theta0_A[o,i]
       b[o]   = theta_d @ P_b + theta0_b
       out    = input @ A.T + b                          # [4096, 1024]

Sharding: out_features (o) split 8 ways -> 128 o per core.  Each core gets its
P_A / theta0_A / P_b / theta0_b shard; input (pre-transposed on host to
[in_f, n]) and theta_d are replicated.  Each core computes out_T shard
[128, 4096]; host concatenates and transposes back.

Per-core dataflow:
  1. einsum: for each o (128): DMA P_A[o] ([d=128, i=1024], 256 KB contiguous),
     then 8 matmuls with the [d,128] slice as the *stationary* operand and
     theta_d [d,1] as the N=1 moving operand -> A_T column [i_local, 1] in
     PSUM.  This streams P_A through the PE at LDWEIGHTS rate (~612 GB/s),
     fully hidden under the ~358 GB/s HBM DMA stream.
  2. A_T[k] = PSUM + theta0_A_T[k]  (DVE add, evacuates PSUM).
  3. main matmul: out_T[:, nb] = sum_k A_T[k].T @ x_T[k, nb], k-inner PSUM
     accumulation, bias fused into the PSUM->SBUF eviction.
"""

from contextlib import ExitStack

import numpy as np

import concourse.bacc as bacc
import concourse.bass as bass
import concourse.mybir as mybir
import concourse.tile as tile
from concourse.bass_utils import run_bass_kernel_spmd

P = 128          # partitions / d / per-core o-shard
IN_F = 1024
OUT_F = 1024
NTOK = 4096
NCORES = 8
KB = IN_F // P   # 8 k-blocks of the contraction dim
FD = 512         # fp32 moving-operand max free dim
NB = NTOK // FD  # 8 n-blocks
F32 = mybir.dt.float32

_CACHE = {}


def _emit_body(nc, tc, ctx, d, pools):
    consts, inp_pool, pa_pool, asb_pool, ps_e, ps_b, ps_o, outsb = pools

    th_sb = consts.tile([P, 1], F32, name="th_sb")
    nc.sync.dma_start(th_sb[:], d["theta"][:, :])
    pb_sb = consts.tile([P, P], F32, name="pb_sb")
    nc.sync.dma_start(pb_sb[:], d["pb"][:, :])
    t0b_sb = consts.tile([P, 1], F32, name="t0b_sb")
    nc.sync.dma_start(t0b_sb[:], d["t0b"][:, :])
    t0a_sb = consts.tile([P, IN_F], F32, name="t0a_sb")
    for k in range(KB):
        nc.sync.dma_start(t0a_sb[:, k * P:(k + 1) * P],
                          d["t0aT"][k * P:(k + 1) * P, :])
    b_sb = consts.tile([P, 1], F32, name="b_sb")

    # resident input (transposed) tiles: x_sb[k] = x_T[k*128:(k+1)*128, :]
    x_sb = []
    for k in range(KB):
        xt = inp_pool.tile([P, NTOK], F32, name=f"x_sb{k}", tag="x_sb")
        nc.sync.dma_start(xt[:], d["xT"][k * P:(k + 1) * P, :])
        x_sb.append(xt)

    # bias: b = P_b.T @ theta + theta0_b     [o, 1]
    bp = ps_b.tile([P, 1], F32, name="bp")
    nc.tensor.matmul(bp[:], lhsT=pb_sb[:], rhs=th_sb[:], start=True, stop=True)
    nc.vector.tensor_add(b_sb[:], bp[:], t0b_sb[:])

    # einsum: A_T columns
    psum_e = [ps_e.tile([P, FD], F32, name=f"psum_e{j}", tag="psum_e")
              for j in range(2)]
    for o in range(P):
        pa_t = pa_pool.tile([P, IN_F], F32, name="pa_t")
        nc.sync.dma_start(pa_t[:], d["pa"][o, :, :])
        for k in range(KB):
            col = (k % 4) * P + o
            nc.tensor.matmul(
                psum_e[k // 4][:, col:col + 1],
                lhsT=pa_t[:, k * P:(k + 1) * P],
                rhs=th_sb[:],
                start=True, stop=True)

    a_sb = asb_pool.tile([P, IN_F], F32, name="a_sb")
    for k in range(KB):
        nc.vector.tensor_add(
            a_sb[:, k * P:(k + 1) * P],
            psum_e[k // 4][:, (k % 4) * P:(k % 4 + 1) * P],
            t0a_sb[:, k * P:(k + 1) * P])

    # main matmul: out_T[:, nb] = sum_k A_T[k].T @ x_T[k][:, nb] ; + b
    for nb in range(NB):
        po = ps_o.tile([P, FD], F32, name="po")
        for k in range(KB):
            nc.tensor.matmul(
                po[:],
                lhsT=a_sb[:, k * P:(k + 1) * P],
                rhs=x_sb[k][:, nb * FD:(nb + 1) * FD],
                start=(k == 0), stop=(k == KB - 1))
        ot = outsb.tile([P, FD], F32, name="ot")
        nc.vector.tensor_scalar_add(ot[:], po[:], b_sb[:, 0:1])
        nc.sync.dma_start(d["out"][:, nb * FD:(nb + 1) * FD], ot[:])


def _build(reps=1):
    nc = bacc.Bacc("TRN2", target_bir_lowering=False, debug=False,
                   num_devices=NCORES)

    d = {
        "xT": nc.dram_tensor("xT", [IN_F, NTOK], F32, kind="ExternalInput"),
        "theta": nc.dram_tensor("theta", [P, 1], F32, kind="ExternalInput"),
        "pa": nc.dram_tensor("pa", [P, P, IN_F], F32, kind="ExternalInput"),
        "t0aT": nc.dram_tensor("t0aT", [IN_F, P], F32, kind="ExternalInput"),
        "pb": nc.dram_tensor("pb", [P, P], F32, kind="ExternalInput"),
        "t0b": nc.dram_tensor("t0b", [P, 1], F32, kind="ExternalInput"),
        "out": nc.dram_tensor("out", [P, NTOK], F32, kind="ExternalOutput"),
    }

    with tile.TileContext(nc) as tc:
        with ExitStack() as ctx:
            pools = (
                ctx.enter_context(tc.tile_pool(name="consts", bufs=2)),
                ctx.enter_context(tc.tile_pool(name="inp", bufs=KB)),
                ctx.enter_context(tc.tile_pool(name="pa", bufs=4)),
                ctx.enter_context(tc.tile_pool(name="asb", bufs=1)),
                ctx.enter_context(tc.tile_pool(name="ps_e", bufs=2,
                                               space="PSUM")),
                ctx.enter_context(tc.tile_pool(name="ps_b", bufs=1,
                                               space="PSUM")),
                ctx.enter_context(tc.tile_pool(name="ps_o", bufs=3,
                                               space="PSUM")),
                ctx.enter_context(tc.tile_pool(name="outsb", bufs=3)),
            )
            for _ in range(reps):
                _emit_body(nc, tc, ctx, d, pools)

    nc.compile()
    return nc


def _in_maps(inputs):
    x = np.asarray(inputs["input"], dtype=np.float32)
    theta_d = np.asarray(inputs["theta_d"], dtype=np.float32)
    theta0_A = np.asarray(inputs["theta0_A"], dtype=np.float32)
    P_A = np.asarray(inputs["P_A"], dtype=np.float32)
    theta0_b = np.asarray(inputs["theta0_b"], dtype=np.float32)
    P_b = np.asarray(inputs["P_b"], dtype=np.float32)

    xT = np.ascontiguousarray(x.T)                    # [in_f, n]
    th = np.ascontiguousarray(theta_d.reshape(P, 1))
    t0aT = np.ascontiguousarray(theta0_A.T)           # [in_f, out_f]

    maps = []
    for c in range(NCORES):
        o0 = c * P
        maps.append({
            "xT": xT,
            "theta": th,
            "pa": np.ascontiguousarray(P_A[o0:o0 + P]),
            "t0aT": np.ascontiguousarray(t0aT[:, o0:o0 + P]),
            "pb": np.ascontiguousarray(P_b[:, o0:o0 + P]),
            "t0b": np.ascontiguousarray(theta0_b[o0:o0 + P].reshape(P, 1)),
        })
    return maps


def run(inputs, trace=False):
    """Returns (output [4096,1024] f32, exec_time_ns or None)."""
    if "nc" not in _CACHE:
        _CACHE["nc"] = _build()
    nc = _CACHE["nc"]
    res = run_bass_kernel_spmd(nc, _in_maps(inputs),
                               core_ids=list(range(NCORES)), trace=trace)
    shards = [res.results[c]["out"] for c in range(NCORES)]   # [128, 4096] each
    outT = np.concatenate(shards, axis=0)                     # [out_f, n]
    return np.ascontiguousarray(outT.T), res.exec_time_ns


def kernel(**inputs):
    out, _ = run(inputs, trace=False)
    return out


# revision 10
# speedup vs baseline: 4.4703x; 4.4703x over previous
"""LinearOffsetLayer Trainium2 kernel (8 NeuronCores, tensor-parallel on out_features).

Math:  A[o,i] = sum_d theta_d[d] * P_A[o,d,i] + theta0_A[o,i]
       b[o]   = theta_d @ P_b + theta0_b
       out    = input @ A.T + b                          # [4096, 1024]

Sharding: out_features (o) split 8 ways -> 128 o per core.  Each core gets its
P_A / theta0_A / P_b / theta0_b shard; input (pre-transposed on host to
[in_f, n]) and theta_d are replicated.  Each core computes out_T shard
[128, 4096]; host concatenates and transposes back.

Per-core dataflow (v2 - streaming einsum):
  1. einsum: theta_d [d,1] is the *stationary* operand (M=1), P_A[o] tiles
     [d=128, i<=512] stream as the moving operand -> PSUM rows
     A_off[o, i-half] = [1, 512].  One matmul per (o, half): 256 matmuls,
     ~213 ns each, hidden under the P_A DMA stream.
  2. Row eviction: [1,512] PSUM->SBUF copies build a_sb [o, i];
     split across DVE (even o) and ACT (odd o).
  3. Transpose: 8 PE transposes a_sb[:, k] -> PSUM; DVE adds theta0_A_T
     during eviction -> aT_sb [i, o].
  4. main matmul: out_T[:, nb] = sum_k aT_sb[k].T @ x_T[k, nb], k-inner PSUM
     accumulation, bias fused into the PSUM->SBUF eviction.
"""

from contextlib import ExitStack

import numpy as np

import concourse.bacc as bacc
import concourse.bass as bass
import concourse.mybir as mybir
import concourse.tile as tile
from concourse.bass_utils import run_bass_kernel_spmd
from concourse.masks import make_identity

P = 128          # partitions / d / per-core o-shard
IN_F = 1024
OUT_F = 1024
NTOK = 4096
NCORES = 8
KB = IN_F // P   # 8 k-blocks of the contraction dim
FD = 512         # fp32 moving-operand max free dim
NH = IN_F // FD  # 2 i-halves per o-row in the einsum
NB = NTOK // FD  # 8 n-blocks
F32 = mybir.dt.float32

PA_GROUP = 1     # o-rows per P_A DMA (chunk = PA_GROUP * 512 KB)
PA_BUFS = 4
MM_DT = "f32r"   # "f32" | "f32r"  - matmul operand dtype for einsum/main

_CACHE = {}


def _mmdt():
    return mybir.dt.float32r if MM_DT == "f32r" else F32


def _emit_body(nc, tc, ctx, d, pools, identity):
    consts, inp_pool, pa_pool, asb_pool, ps_r, ps_o, outsb = pools

    th_sb = consts.tile([P, 1], F32, name="th_sb")
    nc.sync.dma_start(th_sb[:], d["theta"][:, :])
    pb_sb = consts.tile([P, P], F32, name="pb_sb")
    nc.sync.dma_start(pb_sb[:], d["pb"][:, :])
    t0b_sb = consts.tile([P, 1], F32, name="t0b_sb")
    nc.sync.dma_start(t0b_sb[:], d["t0b"][:, :])
    t0a_sb = consts.tile([P, IN_F], F32, name="t0a_sb")
    for k in range(KB):
        nc.sync.dma_start(t0a_sb[:, k * P:(k + 1) * P],
                          d["t0aT"][k * P:(k + 1) * P, :])
    b_sb = consts.tile([P, 1], F32, name="b_sb")

    # resident input (transposed) tiles: x_sb[k] = x_T[k*128:(k+1)*128, :]
    x_sb = []
    for k in range(KB):
        xt = inp_pool.tile([P, NTOK], _mmdt(), name=f"x_sb{k}", tag="x_sb")
        nc.sync.dma_start(xt[:], d["xT"][k * P:(k + 1) * P, :])
        x_sb.append(xt)

    # sliding-window one-hot theta: thwin[d, c] = theta[d] iff c == P-1
    thwin_sb = consts.tile([P, 2 * P - 1], _mmdt(), name="thwin_sb")
    nc.sync.dma_start(thwin_sb[:], d["thwin"][:, :])

    # bias: b = P_b.T @ theta + theta0_b     [o, 1]
    bp = ps_o.tile([P, 1], F32, name="bp", tag="po")
    nc.tensor.matmul(bp[:], lhsT=pb_sb[:], rhs=th_sb[:], start=True, stop=True)
    nc.vector.tensor_add(b_sb[:], bp[:], t0b_sb[:])

    # einsum: A_off[o, i] accumulated row-at-a-time in full-width PSUM.
    # lhsT = thwin[:, P-1-o : 2P-1-o] has theta in column o, zeros elsewhere:
    # out += lhsT.T @ P_A[o] adds theta.T @ P_A[o] into PSUM row o only.
    ablk = [ps_r.tile([P, FD], F32, name=f"ablk{h}", tag="ablk")
            for h in range(NH)]
    for og in range(P // PA_GROUP):
        pa_t = pa_pool.tile([P, PA_GROUP, IN_F], _mmdt(), name="pa_t")
        nc.sync.dma_start(
            pa_t[:],
            d["pa"][og * PA_GROUP:(og + 1) * PA_GROUP, :, :]
            .rearrange("g p i -> p g i"))
        for gi in range(PA_GROUP):
            o = og * PA_GROUP + gi
            for h in range(NH):
                nc.tensor.matmul(
                    ablk[h][:, :],
                    lhsT=thwin_sb[:, P - 1 - o:2 * P - 1 - o],
                    rhs=pa_t[:, gi, h * FD:(h + 1) * FD],
                    start=(o == 0), stop=(o == P - 1))
    a_sb = asb_pool.tile([P, IN_F], F32, name="a_sb")
    for h in range(NH):
        nc.vector.tensor_copy(a_sb[:, h * FD:(h + 1) * FD], ablk[h][:, :])

    # transpose a_sb [o,i] -> aT_sb [i,o] via PE; fold in theta0_A_T
    aT_sb = asb_pool.tile([P, IN_F], _mmdt(), name="aT_sb")
    for k in range(KB):
        pt = ps_o.tile([P, P], F32, name="pt", tag="po")
        nc.tensor.transpose(pt[:], a_sb[:, k * P:(k + 1) * P], identity[:])
        nc.vector.tensor_add(
            aT_sb[:, k * P:(k + 1) * P], pt[:], t0a_sb[:, k * P:(k + 1) * P])

    # main matmul: out_T[:, nb] = sum_k aT_sb[k].T @ x_T[k][:, nb] ; + b
    for nb in range(NB):
        po = ps_o.tile([P, FD], F32, name="po", tag="po")
        for k in range(KB):
            nc.tensor.matmul(
                po[:],
                lhsT=aT_sb[:, k * P:(k + 1) * P],
                rhs=x_sb[k][:, nb * FD:(nb + 1) * FD],
                start=(k == 0), stop=(k == KB - 1))
        ot = outsb.tile([P, FD], F32, name="ot")
        nc.vector.tensor_scalar_add(ot[:], po[:], b_sb[:, 0:1])
        nc.sync.dma_start(d["out"][:, nb * FD:(nb + 1) * FD], ot[:])


def _build(reps=1):
    nc = bacc.Bacc("TRN2", target_bir_lowering=False, debug=False,
                   num_devices=NCORES)

    d = {
        "xT": nc.dram_tensor("xT", [IN_F, NTOK], _mmdt(),
                             kind="ExternalInput"),
        "theta": nc.dram_tensor("theta", [P, 1], F32, kind="ExternalInput"),
        "pa": nc.dram_tensor("pa", [P, P, IN_F], _mmdt(),
                             kind="ExternalInput"),
        "t0aT": nc.dram_tensor("t0aT", [IN_F, P], F32, kind="ExternalInput"),
        "pb": nc.dram_tensor("pb", [P, P], F32, kind="ExternalInput"),
        "t0b": nc.dram_tensor("t0b", [P, 1], F32, kind="ExternalInput"),
        "thwin": nc.dram_tensor("thwin", [P, 2 * P - 1], _mmdt(),
                                kind="ExternalInput"),
        "out": nc.dram_tensor("out", [P, NTOK], F32, kind="ExternalOutput"),
    }

    with tile.TileContext(nc) as tc:
        with ExitStack() as ctx:
            pools = (
                ctx.enter_context(tc.tile_pool(name="consts", bufs=2)),
                ctx.enter_context(tc.tile_pool(name="inp", bufs=KB)),
                ctx.enter_context(tc.tile_pool(name="pa", bufs=PA_BUFS)),
                ctx.enter_context(tc.tile_pool(name="asb", bufs=2)),
                ctx.enter_context(tc.tile_pool(name="ps_r", bufs=2,
                                               space="PSUM")),
                ctx.enter_context(tc.tile_pool(name="ps_o", bufs=3,
                                               space="PSUM")),
                ctx.enter_context(tc.tile_pool(name="outsb", bufs=3)),
            )
            const_pool = pools[0]
            identity = const_pool.tile([P, P], F32, name="identity")
            make_identity(nc, identity)
            for _ in range(reps):
                _emit_body(nc, tc, ctx, d, pools, identity)

    nc.compile()
    return nc


def _in_maps(inputs):
    x = np.asarray(inputs["input"], dtype=np.float32)
    theta_d = np.asarray(inputs["theta_d"], dtype=np.float32)
    theta0_A = np.asarray(inputs["theta0_A"], dtype=np.float32)
    P_A = np.asarray(inputs["P_A"], dtype=np.float32)
    theta0_b = np.asarray(inputs["theta0_b"], dtype=np.float32)
    P_b = np.asarray(inputs["P_b"], dtype=np.float32)

    xT = np.ascontiguousarray(x.T)                    # [in_f, n]
    th = np.ascontiguousarray(theta_d.reshape(P, 1))
    thwin = np.zeros((P, 2 * P - 1), np.float32)
    thwin[:, P - 1] = theta_d
    t0aT = np.ascontiguousarray(theta0_A.T)           # [in_f, out_f]

    maps = []
    for c in range(NCORES):
        o0 = c * P
        maps.append({
            "xT": xT,
            "theta": th,
            "pa": np.ascontiguousarray(P_A[o0:o0 + P]),
            "t0aT": np.ascontiguousarray(t0aT[:, o0:o0 + P]),
            "pb": np.ascontiguousarray(P_b[:, o0:o0 + P]),
            "t0b": np.ascontiguousarray(theta0_b[o0:o0 + P].reshape(P, 1)),
            "thwin": thwin,
        })
    return maps


def run(inputs, trace=False):
    """Returns (output [4096,1024] f32, exec_time_ns or None)."""
    if "nc" not in _CACHE:
        _CACHE["nc"] = _build()
    nc = _CACHE["nc"]
    res = run_bass_kernel_spmd(nc, _in_maps(inputs),
                               core_ids=list(range(NCORES)), trace=trace)
    shards = [res.results[c]["out"] for c in range(NCORES)]   # [128, 4096] each
    outT = np.concatenate(shards, axis=0)                     # [out_f, n]
    return np.ascontiguousarray(outT.T), res.exec_time_ns


def kernel(**inputs):
    out, _ = run(inputs, trace=False)
    return out
